# revision 35
# baseline (speedup 1.0000x reference)
"""Trainium2 Bass kernel for nn_ComparisonLayer (per-o restructure).

Computes, for x:(L,B,D) with L=512,B=2,D=256,C=128,O=64:
    xb  = layernorm(transpose(x,(1,0,2)))          # (B,L,D)
    a   = xb@w1+b1 ; b = xb@w2+b2                  # (B,L,C)
    out[b,l,m,o] = sum_c a[b,l,c]*b[b,m,c]*w3[c,o] + b3[o]
                   + (a@w4)[b,l,o] - (b@w4)[b,m,o] # (B,L,L,O)

Sharding: 8 cores, core k handles batch k//4 and l-block (k%4)*128.

Per-o formulation (o = output channel, 64 iterations):
    out[l, o, m] = (aT * w3[:,o]).T @ bT          # one K=128 N=512 matmul
                 + a4[l, o]                        # per-partition bias in drain
                 + (b3[o] - b4T[o, m])             # rank-1 ones-matmul, 4x
                                                   # row-tiled (concurrent)
Device output layout is (l, (o, m)) in bf16; host transposes to (l, m, o)
and upcasts to fp32.  Engines: PE does 64 main matmuls + 16 groups of 4
concurrent rank-1 matmuls; aw-builds and PSUM drains rotate across
Scalar/Vector/GpSimd; output leaves via 8 x 1MiB HWDGE DMAs.
"""

import numpy as np
import ml_dtypes

import concourse.bacc as bacc
import concourse.bass as bass
import concourse.mybir as mybir
import concourse.tile as tile
from concourse.bass_utils import run_bass_kernel_spmd

L, B, D, C, O = 512, 2, 256, 128, 64
NCORES = 8
LBLK = 128                   # l rows per core
NT = 4                       # full-L tiles of 128 rows
NTL = 5                      # + one xa tile
OBLK = 8                     # o's per output DMA block
NBLK = O // OBLK             # 8 blocks
AWB = 8                      # o's per aw build batch
LN_EPS = 1e-5

F32 = mybir.dt.float32
BF16 = mybir.dt.bfloat16

# packed bf16 weights layout (columns)
WBF_W1 = 0          # [0:256)   w1g halves (h p) c -> p (h c)
WBF_W2 = 256        # [256:512) w2g halves
WBF_W4 = 512        # [512:576) w4 (C, O)
WBF_ID = 576        # [576:704) id128
WBF_N = 704
# packed f32 weights layout
WF_W3 = 0           # [0:64)  w3 (C, O)
WF_B1 = 64          # b1e
WF_B2 = 65          # b2e
WF_B3 = 66          # b3 on partitions 0..63
WF_N = 67


def _build():
    nc = bacc.Bacc("TRN2", target_bir_lowering=False, debug=False)

    xall_d = nc.dram_tensor("xall", (128, NTL, D), F32, kind="ExternalInput")
    wbf_d = nc.dram_tensor("wbf", (128, WBF_N), BF16, kind="ExternalInput")
    wf32_d = nc.dram_tensor("wf32", (128, WF_N), F32, kind="ExternalInput")
    out_d = nc.dram_tensor("out", (LBLK, O * L), BF16, kind="ExternalOutput")

    AX = mybir.AxisListType.X
    ALU = mybir.AluOpType
    ACT = mybir.ActivationFunctionType

    with tile.TileContext(nc) as tc:
        with (
            tc.tile_pool(name="const", bufs=1) as cp,
            tc.tile_pool(name="work", bufs=2) as wp,
            tc.tile_pool(name="aw", bufs=4) as awp,
            tc.tile_pool(name="ob", bufs=3) as obp,
            tc.tile_pool(name="ps", bufs=8, space="PSUM") as pm,
        ):
            pp = pm  # single deep PSUM ring: preamble + main share 8 banks
            # ---------- loads ----------
            xall = cp.tile([128, NTL, D], F32)
            nc.sync.dma_start(xall[:], xall_d[:])
            wbf = cp.tile([128, WBF_N], BF16)
            nc.sync.dma_start(wbf[:], wbf_d[:])
            wf32 = cp.tile([128, WF_N], F32)
            nc.sync.dma_start(wf32[:], wf32_d[:])

            id128 = wbf[:, WBF_ID:WBF_ID + 128]
            w4s = wbf[:, WBF_W4:WBF_W4 + O]

            epsp = cp.tile([128, 1], F32)
            nc.vector.memset(epsp[:], LN_EPS)
            onesb = cp.tile([128, 128], BF16)
            nc.vector.memset(onesb[:], 1.0)
            # dummy 1-col Sqrt: pulls the ACT table load off the LN chain
            tblw = cp.tile([128, 1], F32)
            nc.scalar.activation(tblw[:], epsp[:], ACT.Sqrt)

            # ---------- HAM warm-up burst ----------
            # The PE clock-gate defaults to K=4/8 (1.2 GHz) and needs ~3.4us
            # of sustained matmul activity to open to 2.4 GHz. The PE is
            # otherwise idle during the LN phase, so burn it with dummy
            # back-to-back matmuls to enter the main loop warm.
            last_warm = [None]

            def warm(n, nm):
                # dummy back-to-back matmuls: keep the PE activity monitor
                # from re-throttling to half clock during dependency stalls
                for wi in range(n):
                    wps = pp.tile([128, L], F32, tag="ps", name=f"{nm}{wi}")
                    nc.tensor.matmul(wps[:], onesb[:], wbf[:, 0:L],
                                     start=True, stop=True)
                    last_warm[0] = wps

            warm(16, "wburst")

            # ---------- layernorm (batched over the 5 tiles) ----------
            s5 = wp.tile([128, NTL], F32, tag="s5")
            vs5 = wp.tile([128, NTL], F32, tag="vs5")
            for t in range(NTL):
                nc.vector.tensor_reduce(
                    s5[:, t:t + 1], xall[:, t, :], axis=AX, op=ALU.add
                )
                sq = wp.tile([128, D], F32, tag="sq")
                nc.scalar.activation(
                    sq[:], xall[:, t, :], ACT.Square,
                    accum_out=vs5[:, t:t + 1],
                )
            mu5 = wp.tile([128, NTL], F32, tag="mu5")
            nc.vector.tensor_scalar_mul(mu5[:], s5[:], 1.0 / D)
            musq5 = wp.tile([128, NTL], F32, tag="musq5")
            nc.vector.tensor_tensor(musq5[:], mu5[:], mu5[:], op=ALU.mult)
            var5 = wp.tile([128, NTL], F32, tag="var5")
            nc.vector.scalar_tensor_tensor(
                var5[:], vs5[:], 1.0 / D, musq5[:],
                op0=ALU.mult, op1=ALU.subtract,
            )
            std5 = wp.tile([128, NTL], F32, tag="std5")
            nc.scalar.activation(std5[:], var5[:], ACT.Sqrt, bias=epsp[:])
            rstd5 = wp.tile([128, NTL], F32, tag="rstd5")
            nc.vector.reciprocal(rstd5[:], std5[:])
            nmrs5 = wp.tile([128, NTL], F32, tag="nmrs5")
            nc.vector.scalar_tensor_tensor(
                nmrs5[:], mu5[:], -1.0, rstd5[:], op0=ALU.mult, op1=ALU.mult,
            )
            # xn[:,t,:] = x*rstd - mu*rstd, alternating scalar/vector
            xn = cp.tile([128, NTL, D], BF16)
            for t in range(NTL):
                if t % 2 == 0:
                    nc.scalar.activation(
                        xn[:, t, :], xall[:, t, :], ACT.Identity,
                        bias=nmrs5[:, t:t + 1], scale=rstd5[:, t:t + 1],
                    )
                else:
                    nc.vector.tensor_scalar(
                        xn[:, t, :], xall[:, t, :],
                        rstd5[:, t:t + 1], nmrs5[:, t:t + 1],
                        op0=ALU.mult, op1=ALU.add,
                    )

            # ---------- transposes: xnT[dh, h, t, lj] ----------
            xnT = cp.tile([128, 2, NTL, 128], BF16)
            for t in range(NTL):
                for h in range(2):
                    tp = pp.tile([128, L], BF16, tag="ps")
                    nc.tensor.transpose(
                        tp[:, 0:128], xn[:, t, h * 128:(h + 1) * 128], id128
                    )
                    nc.scalar.copy(xnT[:, h, t, :], tp[:, 0:128])

            warm(2, "wtr")

            # ---------- bT (C, L), then the row-term path ASAP ----------
            bps = pp.tile([C, L], F32, tag="ps")
            for h in range(2):
                nc.tensor.matmul(
                    bps[:], wbf[:, WBF_W2 + h * 128:WBF_W2 + (h + 1) * 128],
                    xnT[:, h, 0:NT, :], start=(h == 0), stop=(h == 1),
                )
            bT_c = cp.tile([C, L], BF16)
            nc.vector.tensor_scalar_add(bT_c[:], bps[:], wf32[:, WF_B2:WF_B2 + 1])
            warm(2, "wbt")

            # rowRep: (b3 - b4T)[o, m] replicated on partitions 0/32/64/96
            # for the 4x row-tiled rank-1 matmuls (HWDGE sync DMAs)
            b4ps = pp.tile([O, L], F32, tag="ps")
            nc.tensor.matmul(b4ps[:], w4s, bT_c[:], start=True, stop=True)
            negb4T = cp.tile([O, L], BF16)
            nc.vector.tensor_scalar(
                negb4T[:], b4ps[:], -1.0, wf32[0:O, WF_B3:WF_B3 + 1],
                op0=ALU.mult, op1=ALU.add,
            )
            rowRep = cp.tile([128, O * L], BF16)
            for r in range(4):
                eng = nc.sync if r % 2 == 0 else nc.scalar
                eng.dma_start(
                    rowRep[32 * r:32 * r + 1, :], negb4T[:, :]
                )
            warm(2, "wrr")

            # ---------- aT (C, 128), a4 (l, o) ----------
            aps = pp.tile([C, L], F32, tag="ps")
            for h in range(2):
                nc.tensor.matmul(
                    aps[:, 0:128], wbf[:, WBF_W1 + h * 128:WBF_W1 + (h + 1) * 128],
                    xnT[:, h, NT, :], start=(h == 0), stop=(h == 1),
                )
            aT_sb = cp.tile([C, 128], F32)
            nc.vector.tensor_scalar_add(
                aT_sb[:], aps[:, 0:128], wf32[:, WF_B1:WF_B1 + 1])
            aT_c = cp.tile([C, 128], BF16)
            nc.vector.tensor_copy(aT_c[:], aT_sb[:])
            warm(2, "wat")

            a4ps = pp.tile([128, L], F32, tag="ps")
            nc.tensor.matmul(a4ps[:, 0:O], aT_c[:], w4s, start=True, stop=True)
            a4sb = cp.tile([128, O], F32)
            nc.scalar.copy(a4sb[:], a4ps[:, 0:O])
            warm(3, "wlp")

            # ---------- main loop: 16 groups of 4 o's ----------
            # gpsimd cannot touch PSUM and its compute ops cost ~2us each;
            # aw is built in batches of 8 o's on the vector engine via a
            # free-dim-broadcast tensor_tensor, and PSUM drains split
            # scalar:vector = 5:3 (scalar ACTIVATE is the cheaper drain).
            def drain(o, dst, ps):
                a4col = a4sb[:, o:o + 1]
                # first 8 o's alternate engines so the ring slots for
                # group 2+ free up fast; steady state 4:4 by half-block
                if (o % 2 == 0) if o < 8 else (o % 8 < 4):
                    nc.scalar.add(dst, ps, a4col)
                else:
                    nc.vector.tensor_scalar_add(dst, ps, a4col)

            # asymmetric output blocks: small first block fires the first
            # DMA early in the ramp; small last block shortens the tail.
            bounds = [0, 4] + list(range(12, 53, OBLK)) + [56, 60, 62, O]
            blk_of = {}
            for bs, be in zip(bounds, bounds[1:]):
                for o in range(bs, be):
                    blk_of[o] = (bs, be)

            # aw blocks built ~4 groups ahead of use so DVE drain backlog
            # never starves the main matmuls (ring of 4 block buffers)
            aw_tiles = {}

            def build_aw_block(k):
                o0 = AWB * k
                t = awp.tile([C, AWB, 128], BF16, tag="aw", name=f"aw{k}")
                a_bc = aT_sb[:].unsqueeze(1).broadcast_to((C, AWB, 128))
                w_bc = wf32[:, WF_W3 + o0:WF_W3 + o0 + AWB] \
                    .unsqueeze(2).broadcast_to((C, AWB, 128))
                nc.vector.tensor_tensor(t[:], a_bc, w_bc, op=ALU.mult)
                aw_tiles[k] = t

            for k in range(2):
                build_aw_block(k)

            ob = None
            for g in range(O // 4):
                if g % 2 == 0:
                    kblk = g // 2 + 2
                    if kblk < O // AWB:
                        build_aw_block(kblk)
                aw8 = aw_tiles[(4 * g) // AWB]
                # rank-1 row-term matmuls FIRST (start=True): they only
                # need the PSUM slot + rowRep, so all 4 are ready together
                # and run concurrently in their row groups; MM1 accumulates.
                # Exception: groups 0-1 run MM1-first since rowRep lands
                # late (~24us) and the PE would otherwise idle.
                pss = []
                mm2_first = g >= 2
                for j in range(4):
                    o = 4 * g + j
                    ps = pm.tile([128, L], F32, tag="ps")
                    pss.append(ps)
                    if mm2_first:
                        nc.tensor.matmul(
                            ps[:], onesb[32 * j:32 * j + 1, :],
                            rowRep[32 * j:32 * j + 1, o * L:(o + 1) * L],
                            start=True, stop=False, tile_position=(32 * j, 0),
                        )
                    else:
                        nc.tensor.matmul(ps[:], aw8[:, o % AWB, :], bT_c[:],
                                         start=True, stop=False)
                for j in range(4):
                    o = 4 * g + j
                    if mm2_first:
                        nc.tensor.matmul(pss[j][:], aw8[:, o % AWB, :],
                                         bT_c[:], start=False, stop=True)
                    else:
                        nc.tensor.matmul(
                            pss[j][:], onesb[32 * j:32 * j + 1, :],
                            rowRep[32 * j:32 * j + 1, o * L:(o + 1) * L],
                            start=False, stop=True, tile_position=(32 * j, 0),
                        )
                for j in range(4):
                    o = 4 * g + j
                    bs, be = blk_of[o]
                    if o == bs:
                        ob = obp.tile([128, OBLK * L], BF16, tag="ob")
                    sl = ob[:, (o - bs) * L:(o - bs + 1) * L]
                    drain(o, sl, pss[j][:])
                    if o == be - 1:
                        nc.sync.dma_start(
                            out_d[:, bs * L:be * L],
                            ob[:, 0:(be - bs) * L],
                        )
                if g == 1:
                    # bridge the rowRep/drain wait before group 2 is ready
                    for _ in range(3):
                        nc.tensor.matmul(last_warm[0][:], onesb[:],
                                         wbf[:, 0:L], start=True, stop=True)

    nc.compile()
    return nc


_CACHE = {}


def _get_nc():
    if "nc" not in _CACHE:
        _CACHE["nc"] = _build()
    return _CACHE["nc"]


def _make_in_maps(x, ln_gamma, ln_beta, w1, b1, w2, b2, w3, b3, w4):
    x = np.ascontiguousarray(x, dtype=np.float32)
    g = np.asarray(ln_gamma, np.float32)
    be = np.asarray(ln_beta, np.float32)
    w1 = np.asarray(w1, np.float32)
    w2 = np.asarray(w2, np.float32)
    # fold the LN affine into the first-layer weights:
    # (xn*g + be) @ w = xn @ (g[:,None]*w) + be @ w
    w1g = g[:, None] * w1
    w2g = g[:, None] * w2
    b1e = (np.asarray(b1, np.float32) + be @ w1).reshape(C, 1)
    b2e = (np.asarray(b2, np.float32) + be @ w2).reshape(C, 1)
    w3c = np.asarray(w3, np.float32)
    w4f = np.asarray(w4, np.float32)
    b3f = np.asarray(b3, np.float32)

    bf = ml_dtypes.bfloat16
    wbf = np.zeros((128, WBF_N), dtype=bf)
    wbf[:, WBF_W1:WBF_W1 + 256] = \
        w1g.reshape(2, 128, C).transpose(1, 0, 2).reshape(128, 256).astype(bf)
    wbf[:, WBF_W2:WBF_W2 + 256] = \
        w2g.reshape(2, 128, C).transpose(1, 0, 2).reshape(128, 256).astype(bf)
    wbf[:, WBF_W4:WBF_W4 + O] = w4f.astype(bf)
    wbf[:, WBF_ID:WBF_ID + 128] = np.eye(128, dtype=np.float32).astype(bf)

    wf32 = np.zeros((128, WF_N), dtype=np.float32)
    wf32[:, WF_W3:WF_W3 + O] = w3c
    wf32[:, WF_B1] = b1e[:, 0]
    wf32[:, WF_B2] = b2e[:, 0]
    wf32[0:O, WF_B3] = b3f

    in_maps = []
    for k in range(NCORES):
        bi, q = k // (NCORES // B), k % (NCORES // B)
        xb = x[:, bi, :]                                   # (L, D)
        xtiles = xb.reshape(NT, 128, D).transpose(1, 0, 2)  # (128, NT, D)
        xa = xb[q * LBLK:(q + 1) * LBLK, :][:, None, :]     # (128, 1, D)
        xall = np.ascontiguousarray(
            np.concatenate([xtiles, xa], axis=1))           # (128, NTL, D)
        in_maps.append({"xall": xall, "wbf": wbf, "wf32": wf32})
    return in_maps


def kernel_run(inputs, trace=False):
    nc = _get_nc()
    in_maps = _make_in_maps(**inputs)
    res = run_bass_kernel_spmd(
        nc, in_maps, core_ids=list(range(NCORES)), trace=trace,
    )
    out = np.empty((B, L, L, O), dtype=np.float32)
    for k in range(NCORES):
        bi, q = k // (NCORES // B), k % (NCORES // B)
        blk = np.asarray(res.results[k]["out"]).astype(np.float32)
        out[bi, q * LBLK:(q + 1) * LBLK] = \
            blk.reshape(LBLK, O, L).transpose(0, 2, 1)
    return out, res


def kernel(**inputs) -> np.ndarray:
    out, _ = kernel_run(inputs, trace=False)
    return out


# revision 38
# speedup vs baseline: 1.1509x; 1.1509x over previous
"""Trainium2 Bass kernel for nn_ComparisonLayer (per-o restructure).

Computes, for x:(L,B,D) with L=512,B=2,D=256,C=128,O=64:
    xb  = layernorm(transpose(x,(1,0,2)))          # (B,L,D)
    a   = xb@w1+b1 ; b = xb@w2+b2                  # (B,L,C)
    out[b,l,m,o] = sum_c a[b,l,c]*b[b,m,c]*w3[c,o] + b3[o]
                   + (a@w4)[b,l,o] - (b@w4)[b,m,o] # (B,L,L,O)

Sharding: 8 cores, core k handles batch k//4 and l-block (k%4)*128.

Per-o formulation (o = output channel, 64 iterations):
    out[l, o, m] = (aT * w3[:,o]).T @ bT          # one K=128 N=512 matmul
                 + a4[l, o]                        # per-partition bias in drain
                 + (b3[o] - b4T[o, m])             # rank-1 ones-matmul, 4x
                                                   # row-tiled (concurrent)
Device output layout is (l, (o, m)) in bf16; host transposes to (l, m, o)
and upcasts to fp32.  Engines: PE does 64 main matmuls + 16 groups of 4
concurrent rank-1 matmuls; aw-builds and PSUM drains rotate across
Scalar/Vector/GpSimd; output leaves via 8 x 1MiB HWDGE DMAs.
"""

import numpy as np
import ml_dtypes

import concourse.bacc as bacc
import concourse.bass as bass
import concourse.mybir as mybir
import concourse.tile as tile
from concourse.bass_utils import run_bass_kernel_spmd

L, B, D, C, O = 512, 2, 256, 128, 64
NCORES = 8
LBLK = 128                   # l rows per core
NT = 4                       # full-L tiles of 128 rows
NTL = 5                      # + one xa tile
OBLK = 8                     # o's per output DMA block
NBLK = O // OBLK             # 8 blocks
AWB = 8                      # o's per aw build batch
LN_EPS = 1e-5

F32 = mybir.dt.float32
BF16 = mybir.dt.bfloat16

# packed bf16 weights layout (columns)
WBF_W1 = 0          # [0:256)   w1g halves (h p) c -> p (h c)
WBF_W2 = 256        # [256:512) w2g halves
WBF_W4 = 512        # [512:576) w4 (C, O)
WBF_ID = 576        # [576:704) id128
WBF_N = 704
# packed f32 weights layout
WF_W3 = 0           # [0:64)  w3 (C, O)
WF_B1 = 64          # b1e
WF_B2 = 65          # b2e
WF_B3 = 66          # b3 on partitions 0..63
WF_N = 67


def _build():
    nc = bacc.Bacc("TRN2", target_bir_lowering=False, debug=False)

    xall_d = nc.dram_tensor("xall", (128, NTL, D), F32, kind="ExternalInput")
    wbf_d = nc.dram_tensor("wbf", (128, WBF_N), BF16, kind="ExternalInput")
    wf32_d = nc.dram_tensor("wf32", (128, WF_N), F32, kind="ExternalInput")
    out_d = nc.dram_tensor("out", (LBLK, O * L), BF16, kind="ExternalOutput")

    AX = mybir.AxisListType.X
    ALU = mybir.AluOpType
    ACT = mybir.ActivationFunctionType

    with tile.TileContext(nc) as tc:
        with (
            tc.tile_pool(name="const", bufs=1) as cp,
            tc.tile_pool(name="work", bufs=2) as wp,
            tc.tile_pool(name="aw", bufs=4) as awp,
            tc.tile_pool(name="ob", bufs=3) as obp,
            tc.tile_pool(name="ps", bufs=8, space="PSUM") as pm,
        ):
            pp = pm  # single deep PSUM ring: preamble + main share 8 banks
            # ---------- loads ----------
            xall = cp.tile([128, NTL, D], F32)
            nc.sync.dma_start(xall[:], xall_d[:])
            wbf = cp.tile([128, WBF_N], BF16)
            nc.sync.dma_start(wbf[:], wbf_d[:])
            wf32 = cp.tile([128, WF_N], F32)
            nc.sync.dma_start(wf32[:], wf32_d[:])

            id128 = wbf[:, WBF_ID:WBF_ID + 128]
            w4s = wbf[:, WBF_W4:WBF_W4 + O]

            epsp = cp.tile([128, 1], F32)
            nc.vector.memset(epsp[:], LN_EPS)
            onesb = cp.tile([128, 128], BF16)
            nc.vector.memset(onesb[:], 1.0)
            # dummy 1-col Sqrt: pulls the ACT table load off the LN chain
            tblw = cp.tile([128, 1], F32)
            nc.scalar.activation(tblw[:], epsp[:], ACT.Sqrt)

            # ---------- HAM warm-up burst ----------
            # The PE clock-gate defaults to K=4/8 (1.2 GHz) and needs ~3.4us
            # of sustained matmul activity to open to 2.4 GHz. The PE is
            # otherwise idle during the LN phase, so burn it with dummy
            # back-to-back matmuls to enter the main loop warm.
            last_warm = [None]

            def warm(n, nm):
                # dummy back-to-back matmuls: keep the PE activity monitor
                # from re-throttling to half clock during dependency stalls
                for wi in range(n):
                    wps = pp.tile([128, L], F32, tag="ps", name=f"{nm}{wi}")
                    nc.tensor.matmul(wps[:], onesb[:], wbf[:, 0:L],
                                     start=True, stop=True)
                    last_warm[0] = wps

            warm(16, "wburst")

            # ---------- layernorm (batched over the 5 tiles) ----------
            s5 = wp.tile([128, NTL], F32, tag="s5")
            vs5 = wp.tile([128, NTL], F32, tag="vs5")
            for t in range(NTL):
                nc.vector.tensor_reduce(
                    s5[:, t:t + 1], xall[:, t, :], axis=AX, op=ALU.add
                )
                sq = wp.tile([128, D], F32, tag="sq")
                nc.scalar.activation(
                    sq[:], xall[:, t, :], ACT.Square,
                    accum_out=vs5[:, t:t + 1],
                )
            mu5 = wp.tile([128, NTL], F32, tag="mu5")
            nc.vector.tensor_scalar_mul(mu5[:], s5[:], 1.0 / D)
            musq5 = wp.tile([128, NTL], F32, tag="musq5")
            nc.vector.tensor_tensor(musq5[:], mu5[:], mu5[:], op=ALU.mult)
            var5 = wp.tile([128, NTL], F32, tag="var5")
            nc.vector.scalar_tensor_tensor(
                var5[:], vs5[:], 1.0 / D, musq5[:],
                op0=ALU.mult, op1=ALU.subtract,
            )
            std5 = wp.tile([128, NTL], F32, tag="std5")
            nc.scalar.activation(std5[:], var5[:], ACT.Sqrt, bias=epsp[:])
            rstd5 = wp.tile([128, NTL], F32, tag="rstd5")
            nc.vector.reciprocal(rstd5[:], std5[:])
            nmrs5 = wp.tile([128, NTL], F32, tag="nmrs5")
            nc.vector.scalar_tensor_tensor(
                nmrs5[:], mu5[:], -1.0, rstd5[:], op0=ALU.mult, op1=ALU.mult,
            )
            # xn[:,t,:] = x*rstd - mu*rstd, alternating scalar/vector
            xn = cp.tile([128, NTL, D], BF16)
            for t in range(NTL):
                if t % 2 == 0:
                    nc.scalar.activation(
                        xn[:, t, :], xall[:, t, :], ACT.Identity,
                        bias=nmrs5[:, t:t + 1], scale=rstd5[:, t:t + 1],
                    )
                else:
                    nc.vector.tensor_scalar(
                        xn[:, t, :], xall[:, t, :],
                        rstd5[:, t:t + 1], nmrs5[:, t:t + 1],
                        op0=ALU.mult, op1=ALU.add,
                    )

            # ---------- transposes: xnT[dh, h, t, lj] ----------
            xnT = cp.tile([128, 2, NTL, 128], BF16)
            for t in range(NTL):
                for h in range(2):
                    tp = pp.tile([128, L], BF16, tag="ps")
                    nc.tensor.transpose(
                        tp[:, 0:128], xn[:, t, h * 128:(h + 1) * 128], id128
                    )
                    nc.scalar.copy(xnT[:, h, t, :], tp[:, 0:128])

            warm(2, "wtr")

            # ---------- bT (C, L), then the row-term path ASAP ----------
            bps = pp.tile([C, L], F32, tag="ps")
            for h in range(2):
                nc.tensor.matmul(
                    bps[:], wbf[:, WBF_W2 + h * 128:WBF_W2 + (h + 1) * 128],
                    xnT[:, h, 0:NT, :], start=(h == 0), stop=(h == 1),
                )
            bT_c = cp.tile([C, L], BF16)
            nc.vector.tensor_scalar_add(bT_c[:], bps[:], wf32[:, WF_B2:WF_B2 + 1])
            warm(2, "wbt")

            # rowRep: (b3 - b4T)[o, m] replicated on partitions 0/32/64/96
            # for the 4x row-tiled rank-1 matmuls (HWDGE sync DMAs)
            b4ps = pp.tile([O, L], F32, tag="ps")
            nc.tensor.matmul(b4ps[:], w4s, bT_c[:], start=True, stop=True)
            # on scalar: the vector engine is backlogged here and negb4T
            # gates the rowRep DMAs (ramp critical path)
            negb4T = cp.tile([O, L], BF16)
            nc.scalar.activation(
                negb4T[:], b4ps[:], ACT.Identity,
                bias=wf32[0:O, WF_B3:WF_B3 + 1], scale=-1.0,
            )
            rowRep = cp.tile([128, O * L], BF16)
            for r in range(4):
                eng = nc.sync if r % 2 == 0 else nc.scalar
                eng.dma_start(
                    rowRep[32 * r:32 * r + 1, :], negb4T[:, :]
                )
            warm(2, "wrr")

            # ---------- aT (C, 128), a4 (l, o) ----------
            aps = pp.tile([C, L], F32, tag="ps")
            for h in range(2):
                nc.tensor.matmul(
                    aps[:, 0:128], wbf[:, WBF_W1 + h * 128:WBF_W1 + (h + 1) * 128],
                    xnT[:, h, NT, :], start=(h == 0), stop=(h == 1),
                )
            aT_sb = cp.tile([C, 128], F32)
            nc.vector.tensor_scalar_add(
                aT_sb[:], aps[:, 0:128], wf32[:, WF_B1:WF_B1 + 1])
            aT_c = cp.tile([C, 128], BF16)
            nc.vector.tensor_copy(aT_c[:], aT_sb[:])
            warm(2, "wat")

            a4ps = pp.tile([128, L], F32, tag="ps")
            nc.tensor.matmul(a4ps[:, 0:O], aT_c[:], w4s, start=True, stop=True)
            a4sb = cp.tile([128, O], F32)
            nc.scalar.copy(a4sb[:], a4ps[:, 0:O])
            warm(3, "wlp")

            # ---------- main loop: 16 groups of 4 o's ----------
            # gpsimd cannot touch PSUM and its compute ops cost ~2us each;
            # aw is built in batches of 8 o's on the vector engine via a
            # free-dim-broadcast tensor_tensor, and PSUM drains split
            # scalar:vector = 5:3 (scalar ACTIVATE is the cheaper drain).
            def drain(o, dst, ps):
                a4col = a4sb[:, o:o + 1]
                # first 8 and last 4 o's alternate engines (fast ring
                # release / fast tail); steady state 4:4 by half-block
                if (o % 2 == 0) if (o < 8 or o >= 60) else (o % 8 < 4):
                    nc.scalar.add(dst, ps, a4col)
                else:
                    nc.vector.tensor_scalar_add(dst, ps, a4col)

            # asymmetric output blocks: small first block fires the first
            # DMA early in the ramp; small last block shortens the tail.
            bounds = [0, 4] + list(range(12, 53, OBLK)) + [56, 60, 62, O]
            blk_of = {}
            for bs, be in zip(bounds, bounds[1:]):
                for o in range(bs, be):
                    blk_of[o] = (bs, be)

            # aw blocks built ~4 groups ahead of use so DVE drain backlog
            # never starves the main matmuls (ring of 4 block buffers)
            aw_tiles = {}

            def build_aw_block(k):
                o0 = AWB * k
                t = awp.tile([C, AWB, 128], BF16, tag="aw", name=f"aw{k}")
                a_bc = aT_sb[:].unsqueeze(1).broadcast_to((C, AWB, 128))
                w_bc = wf32[:, WF_W3 + o0:WF_W3 + o0 + AWB] \
                    .unsqueeze(2).broadcast_to((C, AWB, 128))
                nc.vector.tensor_tensor(t[:], a_bc, w_bc, op=ALU.mult)
                aw_tiles[k] = t

            for k in range(2):
                build_aw_block(k)

            ob = None
            for g in range(O // 4):
                if g % 2 == 0:
                    kblk = g // 2 + 2
                    if kblk < O // AWB:
                        build_aw_block(kblk)
                aw8 = aw_tiles[(4 * g) // AWB]
                # rank-1 row-term matmuls FIRST (start=True): they only
                # need the PSUM slot + rowRep, so all 4 are ready together
                # and run concurrently in their row groups; MM1 accumulates.
                # Exception: groups 0-1 run MM1-first since rowRep lands
                # late (~24us) and the PE would otherwise idle.
                pss = []
                mm2_first = g >= 2
                for j in range(4):
                    o = 4 * g + j
                    ps = pm.tile([128, L], F32, tag="ps")
                    pss.append(ps)
                    if mm2_first:
                        nc.tensor.matmul(
                            ps[:], onesb[32 * j:32 * j + 1, :],
                            rowRep[32 * j:32 * j + 1, o * L:(o + 1) * L],
                            start=True, stop=False, tile_position=(32 * j, 0),
                        )
                    else:
                        nc.tensor.matmul(ps[:], aw8[:, o % AWB, :], bT_c[:],
                                         start=True, stop=False)
                for j in range(4):
                    o = 4 * g + j
                    if mm2_first:
                        nc.tensor.matmul(pss[j][:], aw8[:, o % AWB, :],
                                         bT_c[:], start=False, stop=True)
                    else:
                        nc.tensor.matmul(
                            pss[j][:], onesb[32 * j:32 * j + 1, :],
                            rowRep[32 * j:32 * j + 1, o * L:(o + 1) * L],
                            start=False, stop=True, tile_position=(32 * j, 0),
                        )
                for j in range(4):
                    o = 4 * g + j
                    bs, be = blk_of[o]
                    if o == bs:
                        ob = obp.tile([128, OBLK * L], BF16, tag="ob")
                    sl = ob[:, (o - bs) * L:(o - bs + 1) * L]
                    drain(o, sl, pss[j][:])
                    if o == be - 1:
                        nc.sync.dma_start(
                            out_d[:, bs * L:be * L],
                            ob[:, 0:(be - bs) * L],
                        )


    nc.compile()
    return nc


_CACHE = {}


def _get_nc():
    if "nc" not in _CACHE:
        _CACHE["nc"] = _build()
    return _CACHE["nc"]


def _make_in_maps(x, ln_gamma, ln_beta, w1, b1, w2, b2, w3, b3, w4):
    x = np.ascontiguousarray(x, dtype=np.float32)
    g = np.asarray(ln_gamma, np.float32)
    be = np.asarray(ln_beta, np.float32)
    w1 = np.asarray(w1, np.float32)
    w2 = np.asarray(w2, np.float32)
    # fold the LN affine into the first-layer weights:
    # (xn*g + be) @ w = xn @ (g[:,None]*w) + be @ w
    w1g = g[:, None] * w1
    w2g = g[:, None] * w2
    b1e = (np.asarray(b1, np.float32) + be @ w1).reshape(C, 1)
    b2e = (np.asarray(b2, np.float32) + be @ w2).reshape(C, 1)
    w3c = np.asarray(w3, np.float32)
    w4f = np.asarray(w4, np.float32)
    b3f = np.asarray(b3, np.float32)

    bf = ml_dtypes.bfloat16
    wbf = np.zeros((128, WBF_N), dtype=bf)
    wbf[:, WBF_W1:WBF_W1 + 256] = \
        w1g.reshape(2, 128, C).transpose(1, 0, 2).reshape(128, 256).astype(bf)
    wbf[:, WBF_W2:WBF_W2 + 256] = \
        w2g.reshape(2, 128, C).transpose(1, 0, 2).reshape(128, 256).astype(bf)
    wbf[:, WBF_W4:WBF_W4 + O] = w4f.astype(bf)
    wbf[:, WBF_ID:WBF_ID + 128] = np.eye(128, dtype=np.float32).astype(bf)

    wf32 = np.zeros((128, WF_N), dtype=np.float32)
    wf32[:, WF_W3:WF_W3 + O] = w3c
    wf32[:, WF_B1] = b1e[:, 0]
    wf32[:, WF_B2] = b2e[:, 0]
    wf32[0:O, WF_B3] = b3f

    in_maps = []
    for k in range(NCORES):
        bi, q = k // (NCORES // B), k % (NCORES // B)
        xb = x[:, bi, :]                                   # (L, D)
        xtiles = xb.reshape(NT, 128, D).transpose(1, 0, 2)  # (128, NT, D)
        xa = xb[q * LBLK:(q + 1) * LBLK, :][:, None, :]     # (128, 1, D)
        xall = np.ascontiguousarray(
            np.concatenate([xtiles, xa], axis=1))           # (128, NTL, D)
        in_maps.append({"xall": xall, "wbf": wbf, "wf32": wf32})
    return in_maps


def kernel_run(inputs, trace=False):
    nc = _get_nc()
    in_maps = _make_in_maps(**inputs)
    res = run_bass_kernel_spmd(
        nc, in_maps, core_ids=list(range(NCORES)), trace=trace,
    )
    out = np.empty((B, L, L, O), dtype=np.float32)
    for k in range(NCORES):
        bi, q = k // (NCORES // B), k % (NCORES // B)
        blk = np.asarray(res.results[k]["out"]).astype(np.float32)
        out[bi, q * LBLK:(q + 1) * LBLK] = \
            blk.reshape(LBLK, O, L).transpose(0, 2, 1)
    return out, res


def kernel(**inputs) -> np.ndarray:
    out, _ = kernel_run(inputs, trace=False)
    return out
